# revision 5
# baseline (speedup 1.0000x reference)
"""GRNN (nn_GRNN_71502615544225) Trainium2 kernel, 8-way sharded over train set.

Math: out[b] = sum_n w[b,n]*y[n] / sum_n w[b,n],  w = exp(-||x_b-t_n||^2/(2s^2)).
The per-row factor exp(x_b^2/(2s^2)) cancels in the ratio. Each core computes
exponent[b,n] = x_b.(t_n/s^2) via ONE mixed-dtype K=128 matmul per tile:
stationary = t' in fp16 (duplicated rows), moving = x in bf16 hi/lo. The per-n
term -t_n^2/(2s^2) is folded into the second matmul's constants:
to' = exp(tsq).[y_n | 1].

exp is split across TWO engines per window so the scalar engine is no longer
the serial bottleneck: ACT takes the left part (exact Exp), DVE takes the
right part via a Schraudolph bitcast exp: i16 = round(a*128/ln2 + (16256-C)),
bitcast int16 -> bf16 gives exp(a)*(1+eps), |eps| <= ~3%. DVE carries 3/7 of
the elements (even windows: cols [1536:2048], odd: [512:1536]); measured
end-to-end rel err 1.05e-2 in fp64 emulation (gate 2e-2). The second bf16
matmul accumulates partial weighted sums + weight sums into one PSUM bank
with disjoint PE column tiles per b-slice. Host adds the 8 partials/divides.

Input DMA is spread over the three DMA-capable engines (SP + ACT hwdge
queues, gpsimd swdge queue) so the ~4.3MB of input streams at ~3x22.5GB/s
instead of serializing on qSP; output bands likewise ride 3 queues.
"""
import numpy as np
import ml_dtypes

import concourse.bacc as bacc
import concourse.mybir as mybir
import concourse.tile as tile
from concourse.bass_utils import run_bass_kernel_spmd

F32 = mybir.dt.float32
F16 = mybir.dt.float16
BF = mybir.dt.bfloat16
I16 = mybir.dt.int16

B, D, O, N = 2048, 64, 16, 100000
NCORES = 8
NS = N // NCORES            # 12500 train rows per core
CH = (NS + 127) // 128      # 98 chunks of 128 rows
NSP = CH * 128              # 12544 padded rows
BSL = B // 512              # 4 b-slices of 512
M_SLICES = CH * BSL         # 392 (chunk, b-slice) matmul slices
GRP = 7                     # slices per window pair (4-slice + 3-slice)
NWIN = 2 * (M_SLICES // GRP)        # 112 windows (56 pairs)

# Schraudolph bf16 exp: exp(a) ~= bitcast_bf16(int16(a*A16 + BSH)).
# The DVE's fp32->int16 output convert truncates toward zero (all values
# positive here), so BSH carries a +0.5 round-compensation on top of the
# -5.5 minimax centering.
A16 = float(128.0 / np.log(2.0))
BSH = float(16256.0 - 5.5 + 0.5)

# ACT/DVE column split inside each window (in cols, 512 per slice):
# both windows give ACT cols [0:1024]; DVE takes the rest (1024 even /
# 512 odd = 3/7 of the stream). ACT and DVE write SEPARATE wt tiles --
# a shared tile serializes the two exps via a WAW tile dependency.
ACT_COLS = 1024


def win_of(m):
    g, r = divmod(m, GRP)
    return (2 * g, r) if r < 4 else (2 * g + 1, r - 4)


# t-tile piece sizes in chunks: small early pieces so compute starts early
# and the per-queue DMA streams stay ahead of consumption.
PIECES = [2, 2, 4, 6, 8, 10, 12, 12, 12, 12, 12, 6]
assert sum(PIECES) == CH
POFF = [0]
for _p in PIECES:
    POFF.append(POFF[-1] + _p)

_prog_cache = {}


def build_program(repeat=1):
    if repeat in _prog_cache:
        return _prog_cache[repeat]
    nc = bacc.Bacc("TRN2", target_bir_lowering=False, debug=False,
                   num_devices=NCORES)
    xc_d = nc.dram_tensor("xc", [128, 2048], BF, kind="ExternalInput").ap()
    tq_d = nc.dram_tensor("tq", [128, NSP], F16, kind="ExternalInput").ap()
    to_d = nc.dram_tensor("to", [128, CH * 17], BF, kind="ExternalInput").ap()
    out_d = nc.dram_tensor("out", [113, 512], F32, kind="ExternalOutput").ap()

    with tile.TileContext(nc) as tc:
        with (
            tc.tile_pool(name="const", bufs=1) as cpool,
            tc.tile_pool(name="tqp", bufs=1) as tqpool,
            tc.tile_pool(name="wring", bufs=6) as wpool,
            tc.tile_pool(name="s4pool", bufs=1, space="PSUM") as s4pool,
            tc.tile_pool(name="s3pool", bufs=1, space="PSUM") as s3pool,
            tc.tile_pool(name="apool", bufs=1, space="PSUM") as apool,
        ):
            xb_t = [cpool.tile([128, 512], BF, tag=f"xb{j}", name=f"xb{j}")
                    for j in range(BSL)]
            tq_t = [tqpool.tile([128, np_ * 128], F16, tag=f"tq{k}",
                                name=f"tq{k}")
                    for k, np_ in enumerate(PIECES)]
            to_t = cpool.tile([128, CH * 17], BF)
            junk = cpool.tile([128, 512], BF)   # zeros, for PE warmup
            nc.gpsimd.memset(junk[:], 0.0)

            acc = apool.tile([128, 512], F32)

            # PE warmup: dummy matmuls with no DMA dependencies keep the PE
            # HAM activity window busy while input DMAs land. Results land in
            # acc rows 0:8; the first real start=True accumulation overwrites.
            for _ in range(8):
                nc.tensor.matmul(acc[0:8, :], junk[:, 0:8], junk[:],
                                 start=True, stop=True)

            # ---- input DMA: spread across SP / ACT / Pool queues --------
            # xb tiles split in column halves; per-queue order = need order.
            def xb_half(j, h):
                c0 = h * 256
                return (xb_t[j][:, c0:c0 + 256],
                        xc_d[:, j * 512 + c0:j * 512 + c0 + 256])

            def piece_ap(k):
                w0, w1 = POFF[k] * 128, POFF[k + 1] * 128
                return tq_t[k][:], tq_d[:, w0:w1]

            def to_part(c0, c1):
                return to_t[:, c0 * 17:c1 * 17], to_d[:, c0 * 17:c1 * 17]

            sp, act, gp = nc.sync, nc.scalar, nc.gpsimd
            # SP queue
            for args in (xb_half(0, 0), xb_half(1, 1), xb_half(3, 0),
                         piece_ap(1), piece_ap(4), piece_ap(7),
                         piece_ap(10), to_part(42, 70)):
                sp.dma_start(*args)
            # ACT queue (dispatches run before the first activation)
            for args in (xb_half(0, 1), xb_half(2, 0), xb_half(3, 1),
                         piece_ap(2), piece_ap(5), piece_ap(8),
                         piece_ap(11), to_part(70, CH)):
                act.dma_start(*args)
            # Pool (gpsimd swdge) queue
            for args in (piece_ap(0), xb_half(1, 0), xb_half(2, 1),
                         to_part(0, 14), piece_ap(3), to_part(14, 42),
                         piece_ap(6), piece_ap(9)):
                gp.dma_start(*args)

            def t_slice(i):
                for k in range(len(PIECES)):
                    if i < POFF[k + 1]:
                        kk = i - POFF[k]
                        return tq_t[k][:, kk * 128:(kk + 1) * 128]
                raise AssertionError

            stile = None
            next_mm2 = 0

            total_ch = CH * repeat
            ring = [None] * (NWIN * repeat)
            for m in range(M_SLICES * repeat):
                i, j = divmod(m, BSL)
                i = i % CH
                w, pos = win_of(m)
                nsl = 4 if w % 2 == 0 else 3
                if pos == 0:
                    if nsl == 4:
                        stile = s4pool.tile([128, 4 * 512], F32, tag="s4",
                                            name="s4")
                    else:
                        stile = s3pool.tile([128, 3 * 512], F32, tag="s3",
                                            name="s3")
                ssl = stile[:, pos * 512:(pos + 1) * 512]
                nc.tensor.matmul(
                    ssl, t_slice(i), xb_t[j][:],
                    start=True, stop=True)

                last = m == M_SLICES * repeat - 1
                if pos == nsl - 1:
                    width = nsl * 512
                    dc = width - ACT_COLS
                    wa = wpool.tile([128, 1024], BF, tag="wa")
                    wd = wpool.tile([128, 1024], BF, tag="wd")
                    # left part: exact exp on the scalar engine
                    nc.scalar.activation(
                        wa[:, :ACT_COLS], stile[:, :ACT_COLS],
                        mybir.ActivationFunctionType.Exp)
                    # right part: Schraudolph bitcast exp on DVE
                    nc.vector.tensor_scalar(
                        wd[:, :dc].bitcast(I16), stile[:, ACT_COLS:width],
                        A16, BSH,
                        mybir.AluOpType.mult, mybir.AluOpType.add)
                    ring[w] = (wa, wd)
                    while (next_mm2 < total_ch
                           and win_of(4 * next_mm2 + 3)[0] <= (w - 2
                                if not last else w)):
                        ic = next_mm2
                        icm = ic % CH
                        for j2 in range(BSL):
                            m2 = 4 * ic + j2
                            w2, pos2 = win_of(m2)
                            wa2, wd2 = ring[w2]
                            if pos2 < 2:
                                mov = wa2[:, pos2 * 512:(pos2 + 1) * 512]
                            else:
                                mov = wd2[:, (pos2 - 2) * 512:(pos2 - 1) * 512]
                            nc.tensor.matmul(
                                acc[32 * j2:32 * j2 + 17, :],
                                to_t[:, 17 * icm:17 * icm + 17],
                                mov,
                                start=(ic == 0), stop=(ic == total_ch - 1),
                                tile_position=(0, 32 * j2))
                        next_mm2 += 1

            # copy PSUM->SBUF (DMA cannot read PSUM), then 4 band stores
            # spread over the three DMA queues.
            res = cpool.tile([128, 512], F32)
            nc.vector.tensor_copy(res[0:113, :], acc[0:113, :])
            for j2, eng in zip(range(BSL), (sp, act, gp, sp)):
                eng.dma_start(
                    out_d[32 * j2:32 * j2 + 17, :],
                    res[32 * j2:32 * j2 + 17, :])
    nc.compile()
    _prog_cache[repeat] = nc
    return nc


def _f16(x):
    return np.asarray(x, dtype=np.float16)


def _bf(x):
    return np.asarray(x, dtype=ml_dtypes.bfloat16)


def host_prep(x, train_inputs, train_outputs, spread):
    x = np.asarray(x, np.float32)
    t = np.asarray(train_inputs, np.float32)
    y = np.asarray(train_outputs, np.float32)
    s = np.float32(1.0) / (2.0 * np.float32(spread[0]) ** 2)

    tp = (t * (2.0 * s)).astype(np.float32)          # [N, 64] = t/s^2
    t16 = _f16(tp)
    tsq = (-s * np.einsum("nd,nd->n", t, t)).astype(np.float64)
    f = np.exp(tsq).astype(np.float32)               # fold exp(tsq) into to
    xh = _bf(x)
    xl = _bf(x - xh.astype(np.float32))

    xc = np.zeros((128, 2048), dtype=ml_dtypes.bfloat16)
    xc[0:64] = xh.T
    xc[64:128] = xl.T

    in_maps = []
    for c in range(NCORES):
        n0 = c * NS
        tq = np.zeros((128, NSP), dtype=np.float16)
        tq[0:64, :NS] = t16[n0:n0 + NS].T
        tq[64:128, :NS] = t16[n0:n0 + NS].T
        to = np.zeros((NSP, 17), dtype=np.float32)
        to[:NS, :16] = y[n0:n0 + NS] * f[n0:n0 + NS, None]
        to[:NS, 16] = f[n0:n0 + NS]
        # sbuf layout [p, 17*o+f] with n = 128*o + p
        to_r = _bf(to.reshape(CH, 128, 17).transpose(1, 0, 2).reshape(128, CH * 17))
        in_maps.append({"xc": xc, "tq": tq, "to": to_r})
    return in_maps


def run_cores(in_maps, trace=False, repeat=1, **kw):
    nc = build_program(repeat)
    return run_bass_kernel_spmd(nc, in_maps, list(range(NCORES)),
                                trace=trace, **kw)


def kernel(x, train_inputs, train_outputs, spread):
    in_maps = host_prep(x, train_inputs, train_outputs, spread)
    res = run_cores(in_maps)
    total = np.zeros((17, B), dtype=np.float64)
    for c in range(NCORES):
        o = res.results[c]["out"].astype(np.float64)   # [113, 512]
        for j in range(BSL):
            total[:, 512 * j:512 * (j + 1)] += o[32 * j:32 * j + 17]
    out = (total[:16] / total[16]).T.astype(np.float32)
    return out


# revision 6
# speedup vs baseline: 1.4405x; 1.4405x over previous
"""GRNN (nn_GRNN_71502615544225) Trainium2 kernel, 8-way sharded over train set.

Math: out[b] = sum_n w[b,n]*y[n] / sum_n w[b,n],  w = exp(-||x_b-t_n||^2/(2s^2)).
The per-row factor exp(x_b^2/(2s^2)) cancels in the ratio. Each core computes
exponent[b,n] = x_b.(t_n/s^2) via ONE mixed-dtype K=128 matmul per 512-b slice:
stationary = t' in fp16 (duplicated rows), moving = x in bf16 hi/lo. The per-n
term -t_n^2/(2s^2) is folded into the second matmul's constants:
to' = exp(tsq).[y_n | 1]; the second bf16 matmul accumulates partial weighted
sums + weight sums into one PSUM bank using disjoint PE column tiles per
b-slice. Host adds the 8 core partials and divides.

exp runs on TWO engines in parallel so the scalar engine is not the serial
bottleneck: per 7-slice group, ACT takes slices 0-3 (exact Exp) and DVE takes
slices 4-6 via a Schraudolph bitcast exp (i16 = a*128/ln2 + (16256-5.5),
int16 bits reinterpreted as bf16 => exp(a)*(1+eps), |eps|<=~3%; measured
end-to-end rel err 1.1e-2 vs the 2e-2 gate). Staging uses FOUR single-pair
PSUM tiles (2+2 banks ACT, 2+1 DVE, 1 acc) so each tile's fill->exp->reuse
cycle hides within one group period and the engines never share a wt tile
(a shared tile serializes the exps via a WAW tile dependency).

Input DMA is spread over the three DMA-capable engines (SP + ACT hwdge
queues, gpsimd swdge) so ~4.3MB streams at ~3x22.5GB/s instead of
serializing on qSP. Outputs ride SP+ACT only (a swdge store pays a ~7us
drain at program end).
"""
import numpy as np
import ml_dtypes

import concourse.bacc as bacc
import concourse.mybir as mybir
import concourse.tile as tile
from concourse.bass_utils import run_bass_kernel_spmd

F32 = mybir.dt.float32
F16 = mybir.dt.float16
BF = mybir.dt.bfloat16
I16 = mybir.dt.int16

B, D, O, N = 2048, 64, 16, 100000
NCORES = 8
NS = N // NCORES            # 12500 train rows per core
CH = (NS + 127) // 128      # 98 chunks of 128 rows
NSP = CH * 128              # 12544 padded rows
BSL = B // 512              # 4 b-slices of 512
M_SLICES = CH * BSL         # 392 (chunk, b-slice) matmul slices
GRP = 7                     # slices per group: 4 ACT + 3 DVE
NPAIR = M_SLICES // GRP     # 56 groups

# Schraudolph bf16 exp: exp(a) ~= bitcast_bf16(int16(a*A16 + BSH)).
A16 = float(128.0 / np.log(2.0))
BSH = float(16256.0 - 5.5)

# t-tile piece sizes in chunks: small early pieces so compute starts early
# and the per-queue DMA streams stay ahead of consumption.
PIECES = [2, 2, 4, 6, 8, 10, 12, 12, 12, 12, 12, 6]
assert sum(PIECES) == CH
POFF = [0]
for _p in PIECES:
    POFF.append(POFF[-1] + _p)

_prog_cache = {}


def build_program(repeat=1):
    if repeat in _prog_cache:
        return _prog_cache[repeat]
    nc = bacc.Bacc("TRN2", target_bir_lowering=False, debug=False,
                   num_devices=NCORES)
    xc_d = nc.dram_tensor("xc", [128, 2048], BF, kind="ExternalInput").ap()
    tq_d = nc.dram_tensor("tq", [128, NSP], F16, kind="ExternalInput").ap()
    to_d = nc.dram_tensor("to", [128, CH * 17], BF, kind="ExternalInput").ap()
    out_d = nc.dram_tensor("out", [113, 512], F32, kind="ExternalOutput").ap()

    with tile.TileContext(nc) as tc:
        with (
            tc.tile_pool(name="const", bufs=1) as cpool,
            tc.tile_pool(name="tqp", bufs=1) as tqpool,
            tc.tile_pool(name="wring", bufs=3) as wpool,
            tc.tile_pool(name="pa0", bufs=1, space="PSUM") as pa0,
            tc.tile_pool(name="pa1", bufs=1, space="PSUM") as pa1,
            tc.tile_pool(name="pd0", bufs=1, space="PSUM") as pd0,
            tc.tile_pool(name="pd1", bufs=1, space="PSUM") as pd1,
            tc.tile_pool(name="apool", bufs=1, space="PSUM") as apool,
        ):
            xb_t = [cpool.tile([128, 512], BF, tag=f"xb{j}", name=f"xb{j}")
                    for j in range(BSL)]
            tq_t = [tqpool.tile([128, np_ * 128], F16, tag=f"tq{k}",
                                name=f"tq{k}")
                    for k, np_ in enumerate(PIECES)]
            to_t = cpool.tile([128, CH * 17], BF)
            junk = cpool.tile([128, 512], BF)   # zeros, for PE warmup
            nc.gpsimd.memset(junk[:], 0.0)

            acc = apool.tile([128, 512], F32)

            # PE warmup: dummy matmuls with no DMA dependencies keep the PE
            # HAM activity window busy while input DMAs land. Results land in
            # acc rows 0:8; the first real start=True accumulation overwrites.
            for _ in range(8):
                nc.tensor.matmul(acc[0:8, :], junk[:, 0:8], junk[:],
                                 start=True, stop=True)

            # ---- input DMA: spread across SP / ACT / Pool queues --------
            def xb_half(j, h):
                c0 = h * 256
                return (xb_t[j][:, c0:c0 + 256],
                        xc_d[:, j * 512 + c0:j * 512 + c0 + 256])

            def piece_ap(k):
                w0, w1 = POFF[k] * 128, POFF[k + 1] * 128
                return tq_t[k][:], tq_d[:, w0:w1]

            def to_part(c0, c1):
                return to_t[:, c0 * 17:c1 * 17], to_d[:, c0 * 17:c1 * 17]

            sp, act, gp = nc.sync, nc.scalar, nc.gpsimd
            # SP queue
            for args in (xb_half(0, 0), xb_half(1, 1), xb_half(3, 0),
                         piece_ap(1), piece_ap(4), piece_ap(7),
                         piece_ap(10), to_part(42, 70)):
                sp.dma_start(*args)
            # ACT queue (dispatches run before the first activation)
            for args in (xb_half(0, 1), xb_half(2, 0), xb_half(3, 1),
                         piece_ap(2), piece_ap(5), piece_ap(8),
                         piece_ap(11), to_part(70, CH)):
                act.dma_start(*args)
            # Pool (gpsimd swdge) queue
            for args in (piece_ap(0), xb_half(1, 0), xb_half(2, 1),
                         to_part(0, 14), piece_ap(3), to_part(14, 42),
                         piece_ap(6), piece_ap(9)):
                gp.dma_start(*args)

            def t_slice(i):
                for k in range(len(PIECES)):
                    if i < POFF[k + 1]:
                        kk = i - POFF[k]
                        return tq_t[k][:, kk * 128:(kk + 1) * 128]
                raise AssertionError

            total_ch = CH * repeat
            ring = [None] * (NPAIR * repeat)
            cur = {}
            next_mm2 = 0
            for m in range(M_SLICES * repeat):
                i, j = divmod(m, BSL)
                i = i % CH
                g, r = divmod(m, GRP)
                if r == 0:
                    cur[0] = pa0.tile([128, 1024], F32, tag="pa0", name="pa0")
                elif r == 2:
                    cur[1] = pa1.tile([128, 1024], F32, tag="pa1", name="pa1")
                elif r == 4:
                    cur[2] = pd0.tile([128, 1024], F32, tag="pd0", name="pd0")
                elif r == 6:
                    cur[3] = pd1.tile([128, 512], F32, tag="pd1", name="pd1")
                st = cur[r // 2]
                ssl = st[:, (r % 2) * 512:(r % 2) * 512 + 512]
                nc.tensor.matmul(
                    ssl, t_slice(i), xb_t[j][:],
                    start=True, stop=True)

                last = m == M_SLICES * repeat - 1
                if r in (1, 3):      # ACT windows: slices 0-1 / 2-3
                    wa = wpool.tile([128, 1024], BF, tag=f"wa{r // 2}")
                    nc.scalar.activation(
                        wa[:], st[:],
                        mybir.ActivationFunctionType.Exp)
                    cur[4 + r // 2] = wa
                elif r == 5:         # DVE window: slices 4-5
                    wd = wpool.tile([128, 1024], BF, tag="wd0")
                    nc.vector.tensor_scalar(
                        wd[:].bitcast(I16), st[:], A16, BSH,
                        mybir.AluOpType.mult, mybir.AluOpType.add)
                    cur[6] = wd
                elif r == 6:         # DVE window: slice 6
                    wd = wpool.tile([128, 512], BF, tag="wd1")
                    nc.vector.tensor_scalar(
                        wd[:].bitcast(I16), st[:], A16, BSH,
                        mybir.AluOpType.mult, mybir.AluOpType.add)
                    ring[g] = (cur[4], cur[5], cur[6], wd)
                    # mm2 for chunks fully covered by groups <= g-1: the lag
                    # keeps the in-order PE queue off just-issued exps.
                    while (next_mm2 < total_ch
                           and (4 * next_mm2 + 3) // GRP <= (g - 1
                                if not last else g)):
                        ic = next_mm2
                        icm = ic % CH
                        for j2 in range(BSL):
                            m2 = 4 * ic + j2
                            g2, r2 = divmod(m2, GRP)
                            t4 = ring[g2][min(r2 // 2, 3)]
                            c0 = (r2 % 2) * 512 if r2 < 6 else 0
                            nc.tensor.matmul(
                                acc[32 * j2:32 * j2 + 17, :],
                                to_t[:, 17 * icm:17 * icm + 17],
                                t4[:, c0:c0 + 512],
                                start=(ic == 0), stop=(ic == total_ch - 1),
                                tile_position=(0, 32 * j2))
                        next_mm2 += 1

            # copy PSUM->SBUF (DMA cannot read PSUM), then 4 band stores on
            # the two hwdge queues (swdge stores pay a ~7us drain at exit).
            res = cpool.tile([128, 512], F32)
            nc.vector.tensor_copy(res[0:113, :], acc[0:113, :])
            for j2, eng in zip(range(BSL), (sp, act, sp, act)):
                eng.dma_start(
                    out_d[32 * j2:32 * j2 + 17, :],
                    res[32 * j2:32 * j2 + 17, :])
    nc.compile()
    _prog_cache[repeat] = nc
    return nc


def _f16(x):
    return np.asarray(x, dtype=np.float16)


def _bf(x):
    return np.asarray(x, dtype=ml_dtypes.bfloat16)


def host_prep(x, train_inputs, train_outputs, spread):
    x = np.asarray(x, np.float32)
    t = np.asarray(train_inputs, np.float32)
    y = np.asarray(train_outputs, np.float32)
    s = np.float32(1.0) / (2.0 * np.float32(spread[0]) ** 2)

    tp = (t * (2.0 * s)).astype(np.float32)          # [N, 64] = t/s^2
    t16 = _f16(tp)
    tsq = (-s * np.einsum("nd,nd->n", t, t)).astype(np.float64)
    f = np.exp(tsq).astype(np.float32)               # fold exp(tsq) into to
    xh = _bf(x)
    xl = _bf(x - xh.astype(np.float32))

    xc = np.zeros((128, 2048), dtype=ml_dtypes.bfloat16)
    xc[0:64] = xh.T
    xc[64:128] = xl.T

    in_maps = []
    for c in range(NCORES):
        n0 = c * NS
        tq = np.zeros((128, NSP), dtype=np.float16)
        tq[0:64, :NS] = t16[n0:n0 + NS].T
        tq[64:128, :NS] = t16[n0:n0 + NS].T
        to = np.zeros((NSP, 17), dtype=np.float32)
        to[:NS, :16] = y[n0:n0 + NS] * f[n0:n0 + NS, None]
        to[:NS, 16] = f[n0:n0 + NS]
        # sbuf layout [p, 17*o+f] with n = 128*o + p
        to_r = _bf(to.reshape(CH, 128, 17).transpose(1, 0, 2).reshape(128, CH * 17))
        in_maps.append({"xc": xc, "tq": tq, "to": to_r})
    return in_maps


def run_cores(in_maps, trace=False, repeat=1, **kw):
    nc = build_program(repeat)
    return run_bass_kernel_spmd(nc, in_maps, list(range(NCORES)),
                                trace=trace, **kw)


def kernel(x, train_inputs, train_outputs, spread):
    in_maps = host_prep(x, train_inputs, train_outputs, spread)
    res = run_cores(in_maps)
    total = np.zeros((17, B), dtype=np.float64)
    for c in range(NCORES):
        o = res.results[c]["out"].astype(np.float64)   # [113, 512]
        for j in range(BSL):
            total[:, 512 * j:512 * (j + 1)] += o[32 * j:32 * j + 17]
    out = (total[:16] / total[16]).T.astype(np.float32)
    return out
